# revision 14
# baseline (speedup 1.0000x reference)
"""Trainium2 Bass kernel: attention with relative-position bias.

Reference computation (per sequence, B*T=16 sequences of L=512, D=1024):
    qkv = x @ w_qkv;  q,k,v split;  S = q k^T / sqrt(dh) + rel_bias
    P = softmax(S);   out = (P @ v) @ w_out + b_out

Sharding: data-parallel over the B*T axis — 2 sequences per NeuronCore,
weights replicated. No collectives.

Per-core kernel (all matmuls fp16, accumulation fp32 in PSUM):
  - host pre-transposes x -> xT and pre-casts everything to fp16; the q
    columns of w_qkv are pre-scaled by dh^-0.5.
  - qkT = w_qk^T @ xT   (16 chunk tiles of [128, 512]; chunks 0-7 = q^T
    head-pairs, 8-15 = k^T head-pairs)
  - v   = xT^T @ w_v    (natural layout, stored with a 1.0 column appended
    per head: [128, 16*65] so the PV matmul also produces softmax sums)
  - S^T head-pair-packed: two K=64 matmuls concurrent via tile_position
    row tiling, accumulating into separate PSUM banks
  - P = exp(S^T) * expb  where expb = exp(rel_bias^T) is a host-precomputed
    skewed tile per head ([128, 896]; the s-chunk r bias tile is the slice
    [:, 384-128r : 896-128r] — the bias matrix is Toeplitz)
  - O^T|sums = v_aug^T @ P^T per head (M=65), normalize O^T rows by the
    broadcast reciprocal of the sums row
  - y^T = w_out^T @ O^T + b_out; host transposes back.

The per-sequence phases are software-pipelined at the source level:
sequence s+1's projections (A/B) are interleaved into sequence s's
attention (C), and s's output projection (D) into s+1's attention, so the
TensorE instruction stream has dense work while ACT/DVE run the softmax.
"""

import os
import numpy as np
import ml_dtypes

import concourse.bass as bass
import concourse.mybir as mybir
import concourse.tile as tile
from concourse import bacc, bass_utils

HEADS = 16
MAX_REL = 128
B, T, L, D = 2, 8, 512, 1024
DH = D // HEADS          # 64
N_CORES = 8
SEQS = B * T             # 16
SPC = SEQS // N_CORES    # sequences per core = 2
KC = D // 128            # contraction chunks = 8
LC = L // 128            # sequence chunks = 4
HP = HEADS // 2          # head pairs = 8
EXPB_W = 896             # skewed bias tile width (512 + 3*128)

_F32 = mybir.dt.float32
_F16 = mybir.dt.float16

LAST_EXEC_TIME_NS = None


def _build_program():
    nc = bacc.Bacc("TRN2", debug=False)

    # Per-core DRAM I/O (bf16 unless noted).
    xT_d = nc.dram_tensor("xT", [SPC, 128, KC, L], _F16, kind="ExternalInput")
    wqk_d = nc.dram_tensor("wqk", [16, 128, KC, 128], _F16, kind="ExternalInput")
    wv_d = nc.dram_tensor("wv", [2, 128, KC, 512], _F16, kind="ExternalInput")
    wo_d = nc.dram_tensor("wo", [KC, 128, 8, 128], _F16, kind="ExternalInput")
    expb_d = nc.dram_tensor("expb", [HEADS, 128, EXPB_W], _F16, kind="ExternalInput")
    bo_d = nc.dram_tensor("bo", [128, 8], _F32, kind="ExternalInput")
    yT_d = nc.dram_tensor("yT", [SPC, 128, 8, L], _F32, kind="ExternalOutput")

    with tile.TileContext(nc) as tc:
        with (
            tc.tile_pool(name="const", bufs=1) as const_pool,
            tc.tile_pool(name="wstream", bufs=4) as wstream,
            tc.tile_pool(name="xt", bufs=2) as xt_pool,
            tc.tile_pool(name="qkt", bufs=2) as qkt_pool,
            tc.tile_pool(name="vaug", bufs=2) as vaug_pool,
            tc.tile_pool(name="ptile", bufs=8) as p_pool,
            tc.tile_pool(name="ot", bufs=2) as ot_pool,
            tc.tile_pool(name="norm", bufs=8) as norm_pool,
            tc.tile_pool(name="ysb", bufs=3) as y_pool,
            tc.tile_pool(name="ps_mm", bufs=2, space="PSUM") as ps_mm,
            tc.tile_pool(name="ps_s", bufs=2, space="PSUM") as ps_s,
            tc.tile_pool(name="ps_o", bufs=1, space="PSUM") as ps_o,
        ):
            # ---- constants loaded once per core (SWDGE queue, off the
            # critical HWDGE path) ----
            expb_sb = const_pool.tile([128, HEADS, EXPB_W], _F16)
            nc.gpsimd.dma_start(
                out=expb_sb, in_=expb_d.ap().rearrange("h p u -> p h u")
            )
            wv_sb = const_pool.tile([128, 2, KC, 512], _F16)
            nc.gpsimd.dma_start(out=wv_sb, in_=wv_d.ap().rearrange("n p k c -> p n k c"))
            wo_sb = const_pool.tile([128, KC, 8, 128], _F16)
            nc.gpsimd.dma_start(out=wo_sb, in_=wo_d.ap().rearrange("i p m c -> p i m c"))
            bo_sb = const_pool.tile([128, 8], _F32)
            nc.gpsimd.dma_start(out=bo_sb, in_=bo_d.ap())

            # Per-sequence state (tiles), filled in by the phase generators.
            xt_sb = [None] * SPC
            qkt = [None] * SPC
            vaug = [None] * SPC
            ot = [None] * SPC

            def load_x(s):
                xt_sb[s] = xt_pool.tile([128, KC, L], _F16, name="xt", tag="xt")
                nc.sync.dma_start(out=xt_sb[s], in_=xT_d.ap()[s])

            def phase_a(s):
                """qk^T projection: 16 m-chunk steps."""
                qkt[s] = qkt_pool.tile([128, 16, L], _F16, name="qkt", tag="qkt")
                for m in range(16):
                    wqk_sb = wstream.tile([128, KC, 128], _F16, name="wqk", tag="wqk")
                    nc.sync.dma_start(out=wqk_sb, in_=wqk_d.ap()[m])
                    ps = ps_mm.tile([128, L], _F32, name="ps", tag="ps")
                    for k in range(KC):
                        nc.tensor.matmul(
                            ps,
                            wqk_sb[:, k, :],
                            xt_sb[s][:, k, :],
                            start=(k == 0),
                            stop=(k == KC - 1),
                        )
                    if m % 2 == 0:
                        nc.vector.tensor_copy(out=qkt[s][:, m, :], in_=ps)
                    else:
                        nc.scalar.activation(
                            out=qkt[s][:, m, :], in_=ps,
                            func=mybir.ActivationFunctionType.Copy,
                        )
                    yield

            def phase_b(s):
                """v projection: 8 (lc, nh) steps."""
                vaug[s] = vaug_pool.tile([128, LC, HEADS * 65], _F16, name="vaug", tag="vaug")
                va = vaug[s]
                for lc in range(LC):
                    for nh in range(2):
                        ps = ps_mm.tile([128, 512], _F32, name="ps", tag="ps")
                        for k in range(KC):
                            nc.tensor.matmul(
                                ps,
                                xt_sb[s][:, k, lc * 128:(lc + 1) * 128],
                                wv_sb[:, nh, k, :],
                                start=(k == 0),
                                stop=(k == KC - 1),
                            )
                        dst = bass.AP(
                            tensor=va.tensor,
                            offset=va.offset + lc * (HEADS * 65) + nh * 8 * 65,
                            ap=[va.ap[0], [65, 8], [1, 64]],
                        )
                        nc.vector.tensor_copy(
                            out=dst, in_=ps.rearrange("p (h c) -> p h c", h=8)
                        )
                        if nh == 1:
                            ones_dst = bass.AP(
                                tensor=va.tensor,
                                offset=va.offset + lc * (HEADS * 65) + 64,
                                ap=[va.ap[0], [65, HEADS], [1, 1]],
                            )
                            nc.vector.memset(ones_dst, 1.0)
                        yield

            def phase_c(s):
                """attention: 8 head-pair steps."""
                ot[s] = ot_pool.tile([128, KC, L], _F16, name="ot", tag="ot")
                for hp in range(HP):
                    h0, h1 = 2 * hp, 2 * hp + 1
                    q_tile = qkt[s][:, hp, :]
                    k_tile = qkt[s][:, 8 + hp, :]
                    p_tiles = []
                    for r in range(LC):
                        ps0 = ps_s.tile([128, 512], _F32, name="s0", tag="s0")
                        ps1 = ps_s.tile([128, 512], _F32, name="s1", tag="s1")
                        nc.tensor.matmul(
                            ps0,
                            k_tile[0:64, r * 128:(r + 1) * 128],
                            q_tile[0:64, :],
                            start=True, stop=True,
                        )
                        nc.tensor.matmul(
                            ps1,
                            k_tile[64:128, r * 128:(r + 1) * 128],
                            q_tile[64:128, :],
                            start=True, stop=True,
                            tile_position=(64, 0),
                        )
                        p0 = p_pool.tile([128, 512], _F16, name="p0", tag="p0")
                        p1 = p_pool.tile([128, 512], _F16, name="p1", tag="p1")
                        off = 384 - 128 * r
                        nc.scalar.activation(
                            out=p0, in_=ps0, func=mybir.ActivationFunctionType.Exp
                        )
                        nc.vector.tensor_mul(
                            out=p0, in0=p0, in1=expb_sb[:, h0, off:off + 512]
                        )
                        nc.scalar.activation(
                            out=p1, in_=ps1, func=mybir.ActivationFunctionType.Exp
                        )
                        nc.vector.tensor_mul(
                            out=p1, in0=p1, in1=expb_sb[:, h1, off:off + 512]
                        )
                        p_tiles.append((p0, p1))

                    po0 = ps_o.tile([65, 512], _F32, name="po0", tag="o0")
                    po1 = ps_o.tile([65, 512], _F32, name="po1", tag="o1")
                    for r in range(LC):
                        p0, p1 = p_tiles[r]
                        nc.tensor.matmul(
                            po0,
                            vaug[s][:, r, h0 * 65:h0 * 65 + 65],
                            p0,
                            start=(r == 0), stop=(r == LC - 1),
                        )
                        nc.tensor.matmul(
                            po1,
                            vaug[s][:, r, h1 * 65:h1 * 65 + 65],
                            p1,
                            start=(r == 0), stop=(r == LC - 1),
                        )
                    for idx, po in ((0, po0), (1, po1)):
                        rsum = norm_pool.tile([1, 512], _F32, name="rsum", tag="rsum")
                        nc.vector.reciprocal(out=rsum, in_=po[64:65, :])
                        rb = norm_pool.tile([64, 512], _F32, name="rb", tag="rb")
                        nc.gpsimd.partition_broadcast(rb, rsum)
                        nc.vector.tensor_mul(
                            out=ot[s][idx * 64:(idx + 1) * 64, hp, :],
                            in0=po[0:64, :],
                            in1=rb,
                        )
                    yield

            def phase_d(s):
                """output projection: 8 m-chunk steps."""
                for m in range(8):
                    ps = ps_mm.tile([128, L], _F32, name="ps", tag="ps")
                    for i in range(KC):
                        nc.tensor.matmul(
                            ps,
                            wo_sb[:, i, m, :],
                            ot[s][:, i, :],
                            start=(i == 0),
                            stop=(i == KC - 1),
                        )
                    ysb = y_pool.tile([128, L], _F32, name="ysb", tag="ysb")
                    nc.scalar.activation(
                        out=ysb, in_=ps,
                        func=mybir.ActivationFunctionType.Identity,
                        bias=bo_sb[:, m:m + 1],
                    )
                    nc.sync.dma_start(out=yT_d.ap()[s, :, m, :], in_=ysb)
                    yield

            def drive(gen, n=1):
                if gen is None:
                    return False
                for _ in range(n):
                    try:
                        next(gen)
                    except StopIteration:
                        return False
                return True

            def drain(*gens):
                for g in gens:
                    while drive(g):
                        pass

            # ---- pipelined schedule ----
            load_x(0)
            a0, b0 = phase_a(0), phase_b(0)
            drain(a0, b0)

            load_x(1)
            c0, a1, b1 = phase_c(0), phase_a(1), phase_b(1)
            for _ in range(HP):          # 8 hp steps ; 16 A-steps ; 8 B-steps
                drive(c0)
                drive(a1, 2)
                drive(b1, 1)
            drain(c0, a1, b1)

            d0, c1 = phase_d(0), phase_c(1)
            for _ in range(HP):
                drive(c1)
                drive(d0, 1)
            drain(d0, c1)

            drain(phase_d(1))

    nc.compile()
    return nc


def _host_prep(x, w_qkv, rel_emb, w_out, b_out):
    """Build per-core input maps (bf16 casts, transposes, packing)."""
    bf = np.float16
    scale = DH ** -0.5

    xf = np.asarray(x, np.float32).reshape(SEQS, L, D)
    w_qkv = np.asarray(w_qkv, np.float32)
    rel_emb = np.asarray(rel_emb, np.float32)
    w_out = np.asarray(w_out, np.float32)
    b_out = np.asarray(b_out, np.float32)

    # xT: [seq, 128, KC, L]  (element [p, k, l] = x[seq, l, 128k+p])
    xT = xf.transpose(0, 2, 1).reshape(SEQS, KC, 128, L).transpose(0, 2, 1, 3)
    xT = np.ascontiguousarray(xT).astype(bf)

    # wqk: q columns pre-scaled; pack [m, p, k, c] = w[128k+p, 128m+c]
    wqk = w_qkv[:, :2 * D].copy()
    wqk[:, :D] *= scale
    wqk_p = wqk.reshape(KC, 128, 16, 128).transpose(2, 1, 0, 3)
    wqk_p = np.ascontiguousarray(wqk_p).astype(bf)

    # wv: [n, p, k, c] = w_v[128k+p, 512n+c]
    wv = w_qkv[:, 2 * D:]
    wv_p = wv.reshape(KC, 128, 2, 512).transpose(2, 1, 0, 3)
    wv_p = np.ascontiguousarray(wv_p).astype(bf)

    # wo: [i, p, m, c] = w_out[128i+p, 128m+c]
    wo_p = w_out.reshape(KC, 128, 8, 128)
    wo_p = np.ascontiguousarray(wo_p).astype(bf)

    # expb skewed tiles: expb[h, p, u] = exp(g_h[u - p - 384]),
    # g_h[d] = rel_emb[clip(d, -127, 127) + 127, h]
    u = np.arange(EXPB_W)[None, :]
    p = np.arange(128)[:, None]
    didx = np.clip(u - p - 384, -(MAX_REL - 1), MAX_REL - 1) + (MAX_REL - 1)
    expb = np.exp(rel_emb[didx, :]).transpose(2, 0, 1)  # [h, 128, 896]
    expb = np.ascontiguousarray(expb).astype(bf)

    # b_out packed [p, m] = b_out[128m + p]
    bo_p = np.ascontiguousarray(b_out.reshape(8, 128).T).astype(np.float32)

    shared = {
        "wqk": wqk_p, "wv": wv_p, "wo": wo_p, "expb": expb, "bo": bo_p,
    }
    in_maps = []
    for c in range(N_CORES):
        m = dict(shared)
        m["xT"] = xT[c * SPC:(c + 1) * SPC]
        in_maps.append(m)
    return in_maps


_PROGRAM = None


def kernel(x, w_qkv, rel_emb, w_out, b_out):
    global _PROGRAM, LAST_EXEC_TIME_NS
    if _PROGRAM is None:
        _PROGRAM = _build_program()
    nc = _PROGRAM

    in_maps = _host_prep(x, w_qkv, rel_emb, w_out, b_out)
    trace = bool(int(os.environ.get("TRN_KERNEL_TRACE", "0")))
    res = bass_utils.run_bass_kernel_spmd(
        nc, in_maps, core_ids=list(range(N_CORES)), trace=trace,
    )
    LAST_EXEC_TIME_NS = res.exec_time_ns

    # gather: yT [SPC, 128, 8, L] per core -> y [B, T, L, D]
    y = np.empty((SEQS, L, D), np.float32)
    for c in range(N_CORES):
        yT = np.asarray(res.results[c]["yT"], np.float32)
        for s in range(SPC):
            # [128, 8, L] -> [D, L] -> [L, D]
            y[c * SPC + s] = yT[s].reshape(128, 8, L).transpose(1, 0, 2).reshape(D, L).T
    return y.reshape(B, T, L, D)


# revision 17
# speedup vs baseline: 1.4571x; 1.4571x over previous
"""Trainium2 Bass kernel: attention with relative-position bias.

Reference computation (per sequence, B*T=16 sequences of L=512, D=1024):
    qkv = x @ w_qkv;  q,k,v split;  S = q k^T / sqrt(dh) + rel_bias
    P = softmax(S);   out = (P @ v) @ w_out + b_out

Sharding: data-parallel over the B*T axis — 2 sequences per NeuronCore,
weights replicated. No collectives.

Per-core kernel (all matmuls fp16, accumulation fp32 in PSUM):
  - host pre-transposes x -> xT and pre-casts everything to fp16; the q
    columns of w_qkv are pre-scaled by dh^-0.5.
  - qkT = w_qk^T @ xT   (16 chunk tiles of [128, 512]; chunks 0-7 = q^T
    head-pairs, 8-15 = k^T head-pairs)
  - v   = xT^T @ w_v    (natural layout, stored with a 1.0 column appended
    per head: [128, 16*65] so the PV matmul also produces softmax sums)
  - S^T head-pair-packed: two K=64 matmuls concurrent via tile_position
    row tiling, accumulating into separate PSUM banks
  - P = exp(S^T) * expb  where expb = exp(rel_bias^T) is a host-precomputed
    skewed tile per head ([128, 896]; the s-chunk r bias tile is the slice
    [:, 384-128r : 896-128r] — the bias matrix is Toeplitz)
  - O^T|sums = v_aug^T @ P^T per head (M=65), normalize O^T rows by the
    broadcast reciprocal of the sums row
  - y^T = w_out^T @ O^T + b_out; host transposes back.

The per-sequence phases are software-pipelined at the source level:
sequence s+1's projections (A/B) are interleaved into sequence s's
attention (C), and s's output projection (D) into s+1's attention, so the
TensorE instruction stream has dense work while ACT/DVE run the softmax.
"""

import os
import numpy as np
import ml_dtypes

import concourse.bass as bass
import concourse.mybir as mybir
import concourse.tile as tile
from concourse import bacc, bass_utils

HEADS = 16
MAX_REL = 128
B, T, L, D = 2, 8, 512, 1024
DH = D // HEADS          # 64
N_CORES = 8
SEQS = B * T             # 16
SPC = SEQS // N_CORES    # sequences per core = 2
KC = D // 128            # contraction chunks = 8
LC = L // 128            # sequence chunks = 4
HP = HEADS // 2          # head pairs = 8
EXPB_W = 896             # skewed bias tile width (512 + 3*128)

_F32 = mybir.dt.float32
_F16 = mybir.dt.float16

LAST_EXEC_TIME_NS = None


def _build_program():
    nc = bacc.Bacc("TRN2", debug=False)

    # Per-core DRAM I/O (bf16 unless noted).
    xT_d = nc.dram_tensor("xT", [SPC, 128, KC, L], _F16, kind="ExternalInput")
    wqk_d = nc.dram_tensor("wqk", [16, 128, KC, 128], _F16, kind="ExternalInput")
    wv_d = nc.dram_tensor("wv", [2, 128, KC, 512], _F16, kind="ExternalInput")
    wo_d = nc.dram_tensor("wo", [KC, 128, 8, 128], _F16, kind="ExternalInput")
    expb_d = nc.dram_tensor("expb", [HEADS, 128, EXPB_W], _F16, kind="ExternalInput")
    bo_d = nc.dram_tensor("bo", [128, 8], _F32, kind="ExternalInput")
    yT_d = nc.dram_tensor("yT", [SPC, 128, 8, L], _F32, kind="ExternalOutput")

    with tile.TileContext(nc) as tc:
        with (
            tc.tile_pool(name="const", bufs=1) as const_pool,
            tc.tile_pool(name="wstream", bufs=4) as wstream,
            tc.tile_pool(name="xt", bufs=2) as xt_pool,
            tc.tile_pool(name="qkt", bufs=2) as qkt_pool,
            tc.tile_pool(name="vaug", bufs=2) as vaug_pool,
            tc.tile_pool(name="ptile", bufs=8) as p_pool,
            tc.tile_pool(name="ot", bufs=2) as ot_pool,
            tc.tile_pool(name="norm", bufs=3) as norm_pool,
            tc.tile_pool(name="ysb", bufs=3) as y_pool,
            tc.tile_pool(name="ps_mm", bufs=2, space="PSUM") as ps_mm,
            tc.tile_pool(name="ps_s", bufs=2, space="PSUM") as ps_s,
            tc.tile_pool(name="ps_o", bufs=1, space="PSUM") as ps_o,
        ):
            # ---- constants loaded once per core (SWDGE queue, off the
            # critical HWDGE path) ----
            expb_sb = const_pool.tile([128, HEADS, EXPB_W], _F16)
            nc.gpsimd.dma_start(
                out=expb_sb, in_=expb_d.ap().rearrange("h p u -> p h u")
            )
            wv_sb = const_pool.tile([128, 2, KC, 512], _F16)
            nc.gpsimd.dma_start(out=wv_sb, in_=wv_d.ap().rearrange("n p k c -> p n k c"))
            wo_sb = const_pool.tile([128, KC, 8, 128], _F16)
            nc.gpsimd.dma_start(out=wo_sb, in_=wo_d.ap().rearrange("i p m c -> p i m c"))
            bo_sb = const_pool.tile([128, 8], _F32)
            nc.gpsimd.dma_start(out=bo_sb, in_=bo_d.ap())

            # Per-sequence state (tiles), filled in by the phase generators.
            xt_sb = [None] * SPC
            qkt = [None] * SPC
            vaug = [None] * SPC
            ot = [None] * SPC

            def load_x(s):
                xt_sb[s] = xt_pool.tile([128, KC, L], _F16, name="xt", tag="xt")
                nc.sync.dma_start(out=xt_sb[s], in_=xT_d.ap()[s])

            def phase_a(s):
                """qk^T projection: 16 m-chunk steps."""
                qkt[s] = qkt_pool.tile([128, 16, L], _F16, name="qkt", tag="qkt")
                for m in range(16):
                    wqk_sb = wstream.tile([128, KC, 128], _F16, name="wqk", tag="wqk")
                    nc.sync.dma_start(out=wqk_sb, in_=wqk_d.ap()[m])
                    ps = ps_mm.tile([128, L], _F32, name="ps", tag="ps")
                    for k in range(KC):
                        nc.tensor.matmul(
                            ps,
                            wqk_sb[:, k, :],
                            xt_sb[s][:, k, :],
                            start=(k == 0),
                            stop=(k == KC - 1),
                        )
                    if m % 2 == 0:
                        nc.vector.tensor_copy(out=qkt[s][:, m, :], in_=ps)
                    else:
                        nc.scalar.activation(
                            out=qkt[s][:, m, :], in_=ps,
                            func=mybir.ActivationFunctionType.Copy,
                        )
                    yield

            def phase_b(s):
                """v projection: 8 (lc, nh) steps."""
                vaug[s] = vaug_pool.tile([128, LC, HEADS * 65], _F16, name="vaug", tag="vaug")
                va = vaug[s]
                for lc in range(LC):
                    for nh in range(2):
                        ps = ps_mm.tile([128, 512], _F32, name="ps", tag="ps")
                        for k in range(KC):
                            nc.tensor.matmul(
                                ps,
                                xt_sb[s][:, k, lc * 128:(lc + 1) * 128],
                                wv_sb[:, nh, k, :],
                                start=(k == 0),
                                stop=(k == KC - 1),
                            )
                        dst = bass.AP(
                            tensor=va.tensor,
                            offset=va.offset + lc * (HEADS * 65) + nh * 8 * 65,
                            ap=[va.ap[0], [65, 8], [1, 64]],
                        )
                        nc.vector.tensor_copy(
                            out=dst, in_=ps.rearrange("p (h c) -> p h c", h=8)
                        )
                        if nh == 1:
                            ones_dst = bass.AP(
                                tensor=va.tensor,
                                offset=va.offset + lc * (HEADS * 65) + 64,
                                ap=[va.ap[0], [65, HEADS], [1, 1]],
                            )
                            nc.vector.memset(ones_dst, 1.0)
                        yield

            def phase_c(s):
                """attention: 8 head-pair steps."""
                ot[s] = ot_pool.tile([128, KC, L], _F16, name="ot", tag="ot")
                for hp in range(HP):
                    h0, h1 = 2 * hp, 2 * hp + 1
                    q_tile = qkt[s][:, hp, :]
                    k_tile = qkt[s][:, 8 + hp, :]
                    p_tiles = []
                    for r in range(LC):
                        ps0 = ps_s.tile([128, 512], _F32, name="s0", tag="s0")
                        ps1 = ps_s.tile([128, 512], _F32, name="s1", tag="s1")
                        nc.tensor.matmul(
                            ps0,
                            k_tile[0:64, r * 128:(r + 1) * 128],
                            q_tile[0:64, :],
                            start=True, stop=True,
                        )
                        nc.tensor.matmul(
                            ps1,
                            k_tile[64:128, r * 128:(r + 1) * 128],
                            q_tile[64:128, :],
                            start=True, stop=True,
                            tile_position=(64, 0),
                        )
                        p0 = p_pool.tile([128, 512], _F16, name="p0", tag="p0")
                        p1 = p_pool.tile([128, 512], _F16, name="p1", tag="p1")
                        off = 384 - 128 * r
                        nc.scalar.activation(
                            out=p0, in_=ps0, func=mybir.ActivationFunctionType.Exp
                        )
                        nc.vector.tensor_mul(
                            out=p0, in0=p0, in1=expb_sb[:, h0, off:off + 512]
                        )
                        nc.scalar.activation(
                            out=p1, in_=ps1, func=mybir.ActivationFunctionType.Exp
                        )
                        nc.vector.tensor_mul(
                            out=p1, in0=p1, in1=expb_sb[:, h1, off:off + 512]
                        )
                        p_tiles.append((p0, p1))

                    po0 = ps_o.tile([65, 512], _F32, name="po0", tag="o0")
                    po1 = ps_o.tile([65, 512], _F32, name="po1", tag="o1")
                    for r in range(LC):
                        p0, p1 = p_tiles[r]
                        nc.tensor.matmul(
                            po0,
                            vaug[s][:, r, h0 * 65:h0 * 65 + 65],
                            p0,
                            start=(r == 0), stop=(r == LC - 1),
                        )
                        nc.tensor.matmul(
                            po1,
                            vaug[s][:, r, h1 * 65:h1 * 65 + 65],
                            p1,
                            start=(r == 0), stop=(r == LC - 1),
                        )
                    for idx, po in ((0, po0), (1, po1)):
                        rs_sb = norm_pool.tile([1, 512], _F32, name="rs_sb", tag="rs_sb")
                        nc.vector.tensor_copy(out=rs_sb, in_=po[64:65, :])
                        rsum = norm_pool.tile([1, 512], _F32, name="rsum", tag="rsum")
                        nc.vector.reciprocal_approx_fast(out=rsum, in_=rs_sb)
                        rb = norm_pool.tile([64, 512], _F32, name="rb", tag="rb")
                        nc.gpsimd.partition_broadcast(rb, rsum)
                        nc.vector.tensor_mul(
                            out=ot[s][idx * 64:(idx + 1) * 64, hp, :],
                            in0=po[0:64, :],
                            in1=rb,
                        )
                    yield

            def phase_d(s):
                """output projection: 8 m-chunk steps."""
                for m in range(8):
                    ps = ps_mm.tile([128, L], _F32, name="ps", tag="ps")
                    for i in range(KC):
                        nc.tensor.matmul(
                            ps,
                            wo_sb[:, i, m, :],
                            ot[s][:, i, :],
                            start=(i == 0),
                            stop=(i == KC - 1),
                        )
                    ysb = y_pool.tile([128, L], _F32, name="ysb", tag="ysb")
                    nc.scalar.activation(
                        out=ysb, in_=ps,
                        func=mybir.ActivationFunctionType.Identity,
                        bias=bo_sb[:, m:m + 1],
                    )
                    nc.sync.dma_start(out=yT_d.ap()[s, :, m, :], in_=ysb)
                    yield

            def drive(gen, n=1):
                if gen is None:
                    return False
                for _ in range(n):
                    try:
                        next(gen)
                    except StopIteration:
                        return False
                return True

            def drain(*gens):
                for g in gens:
                    while drive(g):
                        pass

            # ---- pipelined schedule ----
            load_x(0)
            a0, b0 = phase_a(0), phase_b(0)
            drain(a0, b0)

            load_x(1)
            c0, a1, b1 = phase_c(0), phase_a(1), phase_b(1)
            for _ in range(HP):          # 8 hp steps ; 16 A-steps ; 8 B-steps
                drive(c0)
                drive(a1, 2)
                drive(b1, 1)
            drain(c0, a1, b1)

            d0, c1 = phase_d(0), phase_c(1)
            for _ in range(HP):
                drive(c1)
                drive(d0, 1)
            drain(d0, c1)

            drain(phase_d(1))

    nc.compile()
    return nc


def _host_prep(x, w_qkv, rel_emb, w_out, b_out):
    """Build per-core input maps (bf16 casts, transposes, packing)."""
    bf = np.float16
    scale = DH ** -0.5

    xf = np.asarray(x, np.float32).reshape(SEQS, L, D)
    w_qkv = np.asarray(w_qkv, np.float32)
    rel_emb = np.asarray(rel_emb, np.float32)
    w_out = np.asarray(w_out, np.float32)
    b_out = np.asarray(b_out, np.float32)

    # xT: [seq, 128, KC, L]  (element [p, k, l] = x[seq, l, 128k+p])
    xT = xf.transpose(0, 2, 1).reshape(SEQS, KC, 128, L).transpose(0, 2, 1, 3)
    xT = np.ascontiguousarray(xT).astype(bf)

    # wqk: q columns pre-scaled; pack [m, p, k, c] = w[128k+p, 128m+c]
    wqk = w_qkv[:, :2 * D].copy()
    wqk[:, :D] *= scale
    wqk_p = wqk.reshape(KC, 128, 16, 128).transpose(2, 1, 0, 3)
    wqk_p = np.ascontiguousarray(wqk_p).astype(bf)

    # wv: [n, p, k, c] = w_v[128k+p, 512n+c]
    wv = w_qkv[:, 2 * D:]
    wv_p = wv.reshape(KC, 128, 2, 512).transpose(2, 1, 0, 3)
    wv_p = np.ascontiguousarray(wv_p).astype(bf)

    # wo: [i, p, m, c] = w_out[128i+p, 128m+c]
    wo_p = w_out.reshape(KC, 128, 8, 128)
    wo_p = np.ascontiguousarray(wo_p).astype(bf)

    # expb skewed tiles: expb[h, p, u] = exp(g_h[u - p - 384]),
    # g_h[d] = rel_emb[clip(d, -127, 127) + 127, h]
    u = np.arange(EXPB_W)[None, :]
    p = np.arange(128)[:, None]
    didx = np.clip(u - p - 384, -(MAX_REL - 1), MAX_REL - 1) + (MAX_REL - 1)
    expb = np.exp(rel_emb[didx, :]).transpose(2, 0, 1)  # [h, 128, 896]
    expb = np.ascontiguousarray(expb).astype(bf)

    # b_out packed [p, m] = b_out[128m + p]
    bo_p = np.ascontiguousarray(b_out.reshape(8, 128).T).astype(np.float32)

    shared = {
        "wqk": wqk_p, "wv": wv_p, "wo": wo_p, "expb": expb, "bo": bo_p,
    }
    in_maps = []
    for c in range(N_CORES):
        m = dict(shared)
        m["xT"] = xT[c * SPC:(c + 1) * SPC]
        in_maps.append(m)
    return in_maps


_PROGRAM = None


def kernel(x, w_qkv, rel_emb, w_out, b_out):
    global _PROGRAM, LAST_EXEC_TIME_NS
    if _PROGRAM is None:
        _PROGRAM = _build_program()
    nc = _PROGRAM

    in_maps = _host_prep(x, w_qkv, rel_emb, w_out, b_out)
    trace = bool(int(os.environ.get("TRN_KERNEL_TRACE", "0")))
    res = bass_utils.run_bass_kernel_spmd(
        nc, in_maps, core_ids=list(range(N_CORES)), trace=trace,
    )
    LAST_EXEC_TIME_NS = res.exec_time_ns

    # gather: yT [SPC, 128, 8, L] per core -> y [B, T, L, D]
    y = np.empty((SEQS, L, D), np.float32)
    for c in range(N_CORES):
        yT = np.asarray(res.results[c]["yT"], np.float32)
        for s in range(SPC):
            # [128, 8, L] -> [D, L] -> [L, D]
            y[c * SPC + s] = yT[s].reshape(128, 8, L).transpose(1, 0, 2).reshape(D, L).T
    return y.reshape(B, T, L, D)
